# revision 5
# baseline (speedup 1.0000x reference)
"""EnergySNN single-step kernel for Trainium2, 8-core data parallel.

Reference computation (per batch row, D=512, L=3 layers):
    s = 0.5*x
    for i in 0..2:
        fb_in = spikes_h[i+1]            (i<2)   |  readout/||readout||  (i==2)
        ff = s @ W_ff[i].T + b_ff[i]
        fb = fb_in @ W_fb[i].T + b_fb[i]
        a_new = 0.9*dend[i] + 0.1*(ff+fb)
        sm    = 0.9*soma[i]*(1-spikes_h[i]) + 0.1*a_new
        bb    = 0.96*b[i] + 0.04*spikes_h[i]
        spk   = (sm - (0.1 + 1.8*bb)) > 0
        s = spk
    readout_new = 0.9*readout + s @ W_out.T + b_out
    out = [sm(3), spk(3), a_new(3), bb(3), readout_new(1)]  -> [13, B, D]

Strategy: pure data parallel over batch (8192 -> 8 x 1024). All [B,D]
activations/state are held in TRANSPOSED layout [D, B_local] on device so that
the matmul moving operand (rhs, contraction over D on partitions) and the
elementwise state updates share one layout -- no on-device transposes, fully
contiguous DMA. Host does the (cheap) numpy transposes and folds the scalar
prefactors 0.5 (input scale) and 0.1 (=1-ALPHA_A) into the weights.
"""

import numpy as np
import sys

sys.path.insert(0, "/opt/trn_rl_repo")

import concourse.bass as bass
import concourse.bacc as bacc
import concourse.mybir as mybir
from concourse import tile
from concourse.bass_utils import run_bass_kernel_spmd

F32 = mybir.dt.float32
OP = mybir.AluOpType
AF = mybir.ActivationFunctionType

# Problem constants (hardcoded per contract)
B = 8192
D = 512
L = 3
NCORES = 8
BL = B // NCORES          # 1024 batch rows per core
P = 128                   # partitions
KC = D // P               # 4 contraction chunks
MC = D // P               # 4 output-d chunks
NW = 512                  # free-dim chunk width (one PSUM bank of fp32)
NCH = BL // NW            # 2 n-chunks per core

ALPHA_M = np.float32(0.9)
ALPHA_A = np.float32(0.9)
RHO = np.float32(0.96)
BETA = np.float32(1.8)
B0 = np.float32(0.1)
ALPHA_OUT = np.float32(0.9)
EPS = np.float32(1e-12)
ONE_MINUS_AM = np.float32(1.0) - ALPHA_M      # 0.1
ONE_MINUS_AA = np.float32(1.0) - ALPHA_A      # 0.1
ONE_MINUS_RHO = np.float32(1.0) - RHO         # 0.04


def build_program():
    """Build the per-core SPMD Bass/Tile program. Returns (nc, names)."""
    nc = bacc.Bacc("TRN2", target_bir_lowering=False)

    # --- DRAM I/O (per-core shapes, transposed world) ---
    xT = nc.dram_tensor("xT", [D, BL], F32, kind="ExternalInput")
    somaT = nc.dram_tensor("somaT", [L, D, BL], F32, kind="ExternalInput")
    spikesT = nc.dram_tensor("spikesT", [L, D, BL], F32, kind="ExternalInput")
    dendT = nc.dram_tensor("dendT", [L, D, BL], F32, kind="ExternalInput")
    bT = nc.dram_tensor("bT", [L, D, BL], F32, kind="ExternalInput")
    readT = nc.dram_tensor("readT", [D, BL], F32, kind="ExternalInput")
    wffT = nc.dram_tensor("wffT", [L, D, D], F32, kind="ExternalInput")
    wfbT = nc.dram_tensor("wfbT", [L, D, D], F32, kind="ExternalInput")
    woutT = nc.dram_tensor("woutT", [D, D], F32, kind="ExternalInput")
    bcomb = nc.dram_tensor("bcomb", [L, 1, D], F32, kind="ExternalInput")
    boutD = nc.dram_tensor("boutD", [1, D], F32, kind="ExternalInput")
    outT = nc.dram_tensor("outT", [4 * L + 1, D, BL], F32, kind="ExternalOutput")

    with tile.TileContext(nc) as tc:
        with (
            tc.tile_pool(name="wpool", bufs=1) as wp,
            tc.tile_pool(name="spool", bufs=2) as sp,
            tc.tile_pool(name="ppool", bufs=1, space=bass.MemorySpace.PSUM) as pp,
        ):
            # ---- persistent constants / weights ----
            ones = wp.tile([1, NW], F32, tag="ones")
            nc.vector.memset(ones[:], 1.0)
            ones128 = wp.tile([P, 1], F32, tag="ones128")
            nc.vector.memset(ones128[:], 1.0)

            wff_sb = [[wp.tile([P, D], F32, tag=f"wff{i}_{k}", name=f"wff{i}_{k}") for k in range(KC)]
                      for i in range(L)]
            wfb_sb = [[wp.tile([P, D], F32, tag=f"wfb{i}_{k}", name=f"wfb{i}_{k}") for k in range(KC)]
                      for i in range(L)]
            wout_sb = [wp.tile([P, D], F32, tag=f"wout{k}", name=f"wout{k}") for k in range(KC)]
            for i in range(L):
                for k in range(KC):
                    nc.sync.dma_start(wff_sb[i][k][:], wffT[i, k * P:(k + 1) * P, :])
                    nc.sync.dma_start(wfb_sb[i][k][:], wfbT[i, k * P:(k + 1) * P, :])
            for k in range(KC):
                nc.sync.dma_start(wout_sb[k][:], woutT[k * P:(k + 1) * P, :])
            bc_sb = [wp.tile([1, D], F32, tag=f"bc{i}", name=f"bc{i}") for i in range(L)]
            for i in range(L):
                nc.sync.dma_start(bc_sb[i][:], bcomb[i, :, :])
            bo_sb = wp.tile([1, D], F32, tag="bo")
            nc.sync.dma_start(bo_sb[:], boutD[:, :])

            # ---- main loop over n-chunks ----
            for n in range(NCH):
                ns = slice(n * NW, (n + 1) * NW)

                # layer-0 ff rhs: xT tiles
                rhs_ff = []
                for k in range(KC):
                    t = sp.tile([P, NW], F32, tag="xs", bufs=4)
                    nc.sync.dma_start(t[:], xT[k * P:(k + 1) * P, ns])
                    rhs_ff.append(t)

                spk_cur = None  # spikes_h[i] tiles for elementwise of layer i
                for i in range(L):
                    # ---- fb rhs for this layer ----
                    if i + 1 < L:
                        spk_next = []
                        for k in range(KC):
                            t = sp.tile([P, NW], F32, tag="spkh", bufs=8)
                            nc.sync.dma_start(
                                t[:], spikesT[i + 1, k * P:(k + 1) * P, ns])
                            spk_next.append(t)
                        rhs_fb = spk_next
                    else:
                        # normalized readout: nrm over partition dim via PE
                        read_sb = []
                        for k in range(KC):
                            t = sp.tile([P, NW], F32, tag="read", bufs=6)
                            nc.sync.dma_start(t[:], readT[k * P:(k + 1) * P, ns])
                            read_sb.append(t)
                        psum_n = pp.tile([1, NW], F32, tag="pn", bufs=2)
                        for k in range(KC):
                            sq = sp.tile([P, NW], F32, tag="sq", bufs=2)
                            nc.scalar.activation(sq[:], read_sb[k][:], AF.Square)
                            nc.tensor.matmul(psum_n[:], ones128[:, 0:1], sq[:],
                                             start=(k == 0), stop=(k == KC - 1))
                        nrm = sp.tile([1, NW], F32, tag="nrm", bufs=2)
                        nc.scalar.activation(nrm[:], psum_n[:], AF.Sqrt)
                        nrm2 = sp.tile([1, NW], F32, tag="nrm2", bufs=2)
                        nc.vector.tensor_scalar_max(nrm2[:], nrm[:], float(EPS))
                        rn = sp.tile([1, NW], F32, tag="rn", bufs=2)
                        nc.vector.reciprocal(rn[:], nrm2[:])
                        psum_b = pp.tile([P, NW], F32, tag="pb", bufs=2)
                        nc.tensor.matmul(psum_b[:], ones[0:1, 0:P], rn[:],
                                         start=True, stop=True)
                        rhs_fb = []
                        for k in range(KC):
                            t = sp.tile([P, NW], F32, tag="fbin", bufs=6)
                            nc.vector.tensor_mul(t[:], read_sb[k][:], psum_b[:])
                            rhs_fb.append(t)

                    # spikes_h[i] tiles for this layer's elementwise update
                    if i == 0:
                        spk_cur = []
                        for k in range(KC):
                            t = sp.tile([P, NW], F32, tag="spkh", bufs=8)
                            nc.sync.dma_start(t[:], spikesT[0, k * P:(k + 1) * P, ns])
                            spk_cur.append(t)

                    # ---- matmuls + elementwise per output d-chunk ----
                    new_spk = []
                    for m in range(MC):
                        msl = slice(m * P, (m + 1) * P)
                        ps = pp.tile([P, NW], F32, tag="mm", bufs=4)
                        for k in range(KC):
                            nc.tensor.matmul(ps[:], wff_sb[i][k][:, msl],
                                             rhs_ff[k][:], start=(k == 0), stop=False)
                        for k in range(KC):
                            nc.tensor.matmul(ps[:], wfb_sb[i][k][:, msl],
                                             rhs_fb[k][:], start=False, stop=False)
                        nc.tensor.matmul(ps[:], bc_sb[i][0:1, msl], ones[0:1, :],
                                         start=False, stop=True)
                        # ps = 0.1*(ff+fb) + 0.1*(b_ff+b_fb)

                        dend = sp.tile([P, NW], F32, tag="dend", bufs=3)
                        nc.sync.dma_start(dend[:], dendT[i, msl, ns])
                        soma = sp.tile([P, NW], F32, tag="soma", bufs=3)
                        nc.sync.dma_start(soma[:], somaT[i, msl, ns])
                        bst = sp.tile([P, NW], F32, tag="bst", bufs=3)
                        nc.sync.dma_start(bst[:], bT[i, msl, ns])
                        sh = spk_cur[m]

                        # u9 = 0.9*(1 - spikes)
                        u = sp.tile([P, NW], F32, tag="u", bufs=2)
                        nc.scalar.activation(u[:], sh[:], AF.Copy,
                                             bias=float(ALPHA_M), scale=-float(ALPHA_M))
                        # a_new = 0.9*dend + ps
                        anew = sp.tile([P, NW], F32, tag="anew", bufs=3)
                        nc.vector.scalar_tensor_tensor(
                            anew[:], dend[:], float(ALPHA_A), ps[:], OP.mult, OP.add)
                        # m9 = soma * u9
                        m9 = sp.tile([P, NW], F32, tag="m9", bufs=2)
                        nc.gpsimd.tensor_mul(m9[:], soma[:], u[:])
                        # sm = 0.1*a_new + m9
                        smt = sp.tile([P, NW], F32, tag="smt", bufs=3)
                        nc.vector.scalar_tensor_tensor(
                            smt[:], anew[:], float(ONE_MINUS_AM), m9[:], OP.mult, OP.add)
                        # s04 = 0.04*spikes
                        s04 = sp.tile([P, NW], F32, tag="s04", bufs=2)
                        nc.scalar.activation(s04[:], sh[:], AF.Copy,
                                             scale=float(ONE_MINUS_RHO))
                        # bb = 0.96*b + s04
                        bbt = sp.tile([P, NW], F32, tag="bbt", bufs=3)
                        nc.vector.scalar_tensor_tensor(
                            bbt[:], bst[:], float(RHO), s04[:], OP.mult, OP.add)
                        # v = -1.8*bb + sm ; spk = v > 0.1
                        v = sp.tile([P, NW], F32, tag="v", bufs=2)
                        nc.vector.scalar_tensor_tensor(
                            v[:], bbt[:], -float(BETA), smt[:], OP.mult, OP.add)
                        spk = sp.tile([P, NW], F32, tag="spk", bufs=8)
                        nc.vector.tensor_single_scalar(spk[:], v[:], float(B0), OP.is_gt)

                        nc.sync.dma_start(outT[i, msl, ns], smt[:])
                        nc.sync.dma_start(outT[L + i, msl, ns], spk[:])
                        nc.sync.dma_start(outT[2 * L + i, msl, ns], anew[:])
                        nc.sync.dma_start(outT[3 * L + i, msl, ns], bbt[:])
                        new_spk.append(spk)

                    rhs_ff = new_spk
                    if i + 1 < L:
                        spk_cur = spk_next

                # ---- readout update: 0.9*readout + spk2 @ W_out.T + b_out ----
                for m in range(MC):
                    msl = slice(m * P, (m + 1) * P)
                    psr = pp.tile([P, NW], F32, tag="mm", bufs=4)
                    for k in range(KC):
                        nc.tensor.matmul(psr[:], wout_sb[k][:, msl], rhs_ff[k][:],
                                         start=(k == 0), stop=False)
                    nc.tensor.matmul(psr[:], bo_sb[0:1, msl], ones[0:1, :],
                                     start=False, stop=True)
                    routt = sp.tile([P, NW], F32, tag="rout", bufs=2)
                    nc.vector.scalar_tensor_tensor(
                        routt[:], read_sb[m][:], float(ALPHA_OUT), psr[:],
                        OP.mult, OP.add)
                    nc.sync.dma_start(outT[4 * L, msl, ns], routt[:])

    nc.compile()
    return nc


def make_in_maps(x, soma, spikes_h, dendrites, b, readout,
                 W_ff, b_ff, W_fb, b_fb, W_out, b_out):
    """Shard + transpose inputs; fold scalar prefactors into weights."""
    f32 = np.float32
    x = np.asarray(x, f32)
    soma = np.asarray(soma, f32)
    spikes_h = np.asarray(spikes_h, f32)
    dendrites = np.asarray(dendrites, f32)
    b = np.asarray(b, f32)
    readout = np.asarray(readout, f32)
    W_ff = np.asarray(W_ff, f32)
    b_ff = np.asarray(b_ff, f32)
    W_fb = np.asarray(W_fb, f32)
    b_fb = np.asarray(b_fb, f32)
    W_out = np.asarray(W_out, f32)
    b_out = np.asarray(b_out, f32)

    # weights, replicated: fold 0.1 (=1-ALPHA_A) and the 0.5 input scale
    wffT = np.stack([
        np.ascontiguousarray((W_ff[i] * (ONE_MINUS_AA * (f32(0.5) if i == 0 else f32(1.0)))).T)
        for i in range(L)
    ])
    wfbT = np.stack([np.ascontiguousarray((W_fb[i] * ONE_MINUS_AA).T) for i in range(L)])
    woutT = np.ascontiguousarray(W_out.T)
    bcomb = (ONE_MINUS_AA * (b_ff + b_fb)).reshape(L, 1, D)
    boutD = b_out.reshape(1, D)

    in_maps = []
    for c in range(NCORES):
        sl = slice(c * BL, (c + 1) * BL)
        in_maps.append({
            "xT": np.ascontiguousarray(x[sl].T),
            "somaT": np.ascontiguousarray(soma[:, sl, :].transpose(0, 2, 1)),
            "spikesT": np.ascontiguousarray(spikes_h[:, sl, :].transpose(0, 2, 1)),
            "dendT": np.ascontiguousarray(dendrites[:, sl, :].transpose(0, 2, 1)),
            "bT": np.ascontiguousarray(b[:, sl, :].transpose(0, 2, 1)),
            "readT": np.ascontiguousarray(readout[sl].T),
            "wffT": wffT,
            "wfbT": wfbT,
            "woutT": woutT,
            "bcomb": np.ascontiguousarray(bcomb),
            "boutD": np.ascontiguousarray(boutD),
        })
    return in_maps


_CACHE = {}


def _get_program():
    if "nc" not in _CACHE:
        _CACHE["nc"] = build_program()
    return _CACHE["nc"]


def kernel(**inputs):
    nc = _get_program()
    in_maps = make_in_maps(**inputs)
    res = run_bass_kernel_spmd(nc, in_maps, core_ids=list(range(NCORES)))
    out = np.empty((4 * L + 1, B, D), np.float32)
    for c in range(NCORES):
        sl = slice(c * BL, (c + 1) * BL)
        out[:, sl, :] = res.results[c]["outT"].transpose(0, 2, 1)
    return out


# revision 7
# speedup vs baseline: 1.0921x; 1.0921x over previous
"""EnergySNN single-step kernel for Trainium2, 8-core data parallel.

Reference computation (per batch row, D=512, L=3 layers):
    s = 0.5*x
    for i in 0..2:
        fb_in = spikes_h[i+1]            (i<2)   |  readout/||readout||  (i==2)
        ff = s @ W_ff[i].T + b_ff[i]
        fb = fb_in @ W_fb[i].T + b_fb[i]
        a_new = 0.9*dend[i] + 0.1*(ff+fb)
        sm    = 0.9*soma[i]*(1-spikes_h[i]) + 0.1*a_new
        bb    = 0.96*b[i] + 0.04*spikes_h[i]
        spk   = (sm - (0.1 + 1.8*bb)) > 0
        s = spk
    readout_new = 0.9*readout + s @ W_out.T + b_out
    out = [sm(3), spk(3), a_new(3), bb(3), readout_new(1)]  -> [13, B, D]

Strategy: pure data parallel over batch (8192 -> 8 x 1024). All [B,D]
activations/state are held in TRANSPOSED layout [D, B_local] on device so that
the matmul moving operand (rhs, contraction over D on partitions) and the
elementwise state updates share one layout -- no on-device transposes, fully
contiguous DMA. Host does the (cheap) numpy transposes and folds the scalar
prefactors 0.5 (input scale) and 0.1 (=1-ALPHA_A) into the weights.

fp32 matmul runs at 4 PE-cycles/row (two half-rate passes). For the 5 GEMMs
whose moving operand is exact in bf16 (spike vectors in {0,1}), the fp32
weights are split exactly into three bf16 matrices (W = W1+W2+W3 covering all
24 mantissa bits); bf16xbf16 products are exact and accumulate in fp32 PSUM,
giving fp32-accurate results at 3 cycles/row. Spikes are moved as bf16
(exact), halving their DMA traffic.
"""

import numpy as np
import sys

sys.path.insert(0, "/opt/trn_rl_repo")

import concourse.bass as bass
import concourse.bacc as bacc
import concourse.mybir as mybir
from concourse import tile
from concourse.bass_utils import run_bass_kernel_spmd

F32 = mybir.dt.float32
BF16 = mybir.dt.bfloat16
NP_BF16 = mybir.dt.np(BF16)
OP = mybir.AluOpType
AF = mybir.ActivationFunctionType

# Problem constants (hardcoded per contract)
B = 8192
D = 512
L = 3
NCORES = 8
BL = B // NCORES          # 1024 batch rows per core
P = 128                   # partitions
KC = D // P               # 4 contraction chunks
MC = D // P               # 4 output-d chunks
NW = 512                  # free-dim chunk width (one PSUM bank of fp32)
NCH = BL // NW            # 2 n-chunks per core
NS = 3                    # bf16 splits per fp32 weight

ALPHA_M = np.float32(0.9)
ALPHA_A = np.float32(0.9)
RHO = np.float32(0.96)
BETA = np.float32(1.8)
B0 = np.float32(0.1)
ALPHA_OUT = np.float32(0.9)
EPS = np.float32(1e-12)
ONE_MINUS_AM = np.float32(1.0) - ALPHA_M      # 0.1
ONE_MINUS_AA = np.float32(1.0) - ALPHA_A      # 0.1
ONE_MINUS_RHO = np.float32(1.0) - RHO         # 0.04


def build_program(use_bias=False):
    """Build the per-core SPMD Bass/Tile program."""
    nc = bacc.Bacc("TRN2", target_bir_lowering=False)

    # --- DRAM I/O (per-core shapes, transposed world) ---
    xT = nc.dram_tensor("xT", [D, BL], F32, kind="ExternalInput")
    somaT = nc.dram_tensor("somaT", [L, D, BL], F32, kind="ExternalInput")
    spikesT = nc.dram_tensor("spikesT", [L, D, BL], BF16, kind="ExternalInput")
    dendT = nc.dram_tensor("dendT", [L, D, BL], F32, kind="ExternalInput")
    bT = nc.dram_tensor("bT", [L, D, BL], F32, kind="ExternalInput")
    readT = nc.dram_tensor("readT", [D, BL], F32, kind="ExternalInput")
    # fp32 weights: layer-0 ff (x rhs), layer-2 fb (normalized-readout rhs)
    wff0T = nc.dram_tensor("wff0T", [D, D], F32, kind="ExternalInput")
    wfb2T = nc.dram_tensor("wfb2T", [D, D], F32, kind="ExternalInput")
    # bf16 3-way exact splits: ff layers 1,2 / fb layers 0,1 / out
    wff3 = nc.dram_tensor("wff3", [2, NS, D, D], BF16, kind="ExternalInput")
    wfb3 = nc.dram_tensor("wfb3", [2, NS, D, D], BF16, kind="ExternalInput")
    wout3 = nc.dram_tensor("wout3", [NS, D, D], BF16, kind="ExternalInput")
    bcomb = nc.dram_tensor("bcomb", [L, 1, D], F32, kind="ExternalInput")
    boutD = nc.dram_tensor("boutD", [1, D], F32, kind="ExternalInput")
    # f32 outputs: sm(0-2), a_new(3-5), bb(6-8), readout_new(9)
    outT = nc.dram_tensor("outT", [3 * L + 1, D, BL], F32, kind="ExternalOutput")
    # spikes out, bf16 (exact 0/1)
    outSpkT = nc.dram_tensor("outSpkT", [L, D, BL], BF16, kind="ExternalOutput")

    with tile.TileContext(nc) as tc:
        with (
            tc.tile_pool(name="wpool", bufs=1) as wp,
            tc.tile_pool(name="spool", bufs=2) as sp,
            tc.tile_pool(name="ppool", bufs=1, space=bass.MemorySpace.PSUM) as pp,
        ):
            # ---- persistent constants / weight tiles ----
            ones = wp.tile([1, NW], F32, tag="ones")
            nc.vector.memset(ones[:], 1.0)
            ones128 = wp.tile([P, 1], F32, tag="ones128")
            nc.vector.memset(ones128[:], 1.0)

            wff0_sb = [wp.tile([P, D], F32, tag=f"wff0_{k}", name=f"wff0_{k}")
                       for k in range(KC)]
            wfb2_sb = [wp.tile([P, D], F32, tag=f"wfb2_{k}", name=f"wfb2_{k}")
                       for k in range(KC)]
            wff3_sb = [[[wp.tile([P, D], BF16, tag=f"wff3_{li}_{s}_{k}",
                                 name=f"wff3_{li}_{s}_{k}")
                         for k in range(KC)] for s in range(NS)] for li in range(2)]
            wfb3_sb = [[[wp.tile([P, D], BF16, tag=f"wfb3_{li}_{s}_{k}",
                                 name=f"wfb3_{li}_{s}_{k}")
                         for k in range(KC)] for s in range(NS)] for li in range(2)]
            wout3_sb = [[wp.tile([P, D], BF16, tag=f"wout3_{s}_{k}",
                                 name=f"wout3_{s}_{k}")
                         for k in range(KC)] for s in range(NS)]
            bc_sb = [wp.tile([1, D], F32, tag=f"bc{i}", name=f"bc{i}")
                     for i in range(L)]
            bo_sb = wp.tile([1, D], F32, tag="bo")

            def load_weights(i):
                """DMA layer i's weights (emitted just before first use)."""
                for k in range(KC):
                    ksl = slice(k * P, (k + 1) * P)
                    if i == 0:
                        nc.sync.dma_start(wff0_sb[k][:], wff0T[ksl, :])
                        for s in range(NS):
                            nc.sync.dma_start(wfb3_sb[0][s][k][:], wfb3[0, s, ksl, :])
                    elif i == 1:
                        for s in range(NS):
                            nc.sync.dma_start(wff3_sb[0][s][k][:], wff3[0, s, ksl, :])
                            nc.sync.dma_start(wfb3_sb[1][s][k][:], wfb3[1, s, ksl, :])
                    else:
                        nc.sync.dma_start(wfb2_sb[k][:], wfb2T[ksl, :])
                        for s in range(NS):
                            nc.sync.dma_start(wff3_sb[1][s][k][:], wff3[1, s, ksl, :])
                if use_bias:
                    nc.sync.dma_start(bc_sb[i][:], bcomb[i, :, :])

            # ---- main loop over n-chunks ----
            for n in range(NCH):
                ns = slice(n * NW, (n + 1) * NW)

                # layer-0 ff rhs: xT tiles (fp32)
                rhs_ff = []
                for k in range(KC):
                    t = sp.tile([P, NW], F32, tag="xs", bufs=4)
                    nc.sync.dma_start(t[:], xT[k * P:(k + 1) * P, ns])
                    rhs_ff.append(t)

                spk_cur = None  # spikes_h[i] tiles (bf16) for layer i elementwise
                for i in range(L):
                    if n == 0:
                        load_weights(i)
                    # ---- fb rhs for this layer ----
                    if i + 1 < L:
                        spk_next = []
                        for k in range(KC):
                            t = sp.tile([P, NW], BF16, tag="spkh", bufs=8)
                            nc.sync.dma_start(
                                t[:], spikesT[i + 1, k * P:(k + 1) * P, ns])
                            spk_next.append(t)
                        rhs_fb = spk_next
                    else:
                        # normalized readout: nrm over partition dim via PE
                        read_sb = []
                        for k in range(KC):
                            t = sp.tile([P, NW], F32, tag="read", bufs=6)
                            nc.sync.dma_start(t[:], readT[k * P:(k + 1) * P, ns])
                            read_sb.append(t)
                        psum_n = pp.tile([1, NW], F32, tag="pn", bufs=2)
                        for k in range(KC):
                            sq = sp.tile([P, NW], F32, tag="sq", bufs=2)
                            nc.scalar.activation(sq[:], read_sb[k][:], AF.Square)
                            nc.tensor.matmul(psum_n[:], ones128[:, 0:1], sq[:],
                                             start=(k == 0), stop=(k == KC - 1))
                        nrm = sp.tile([1, NW], F32, tag="nrm", bufs=2)
                        nc.scalar.activation(nrm[:], psum_n[:], AF.Sqrt)
                        nrm2 = sp.tile([1, NW], F32, tag="nrm2", bufs=2)
                        nc.vector.tensor_scalar_max(nrm2[:], nrm[:], float(EPS))
                        rn = sp.tile([1, NW], F32, tag="rn", bufs=2)
                        nc.vector.reciprocal(rn[:], nrm2[:])
                        psum_b = pp.tile([P, NW], F32, tag="pb", bufs=2)
                        nc.tensor.matmul(psum_b[:], ones[0:1, 0:P], rn[:],
                                         start=True, stop=True)
                        rhs_fb = []
                        for k in range(KC):
                            t = sp.tile([P, NW], F32, tag="fbin", bufs=6)
                            nc.vector.tensor_mul(t[:], read_sb[k][:], psum_b[:])
                            rhs_fb.append(t)

                    # spikes_h[i] tiles for this layer's elementwise update
                    if i == 0:
                        spk_cur = []
                        for k in range(KC):
                            t = sp.tile([P, NW], BF16, tag="spkh", bufs=8)
                            nc.sync.dma_start(t[:], spikesT[0, k * P:(k + 1) * P, ns])
                            spk_cur.append(t)

                    # ---- matmuls + elementwise per output d-chunk ----
                    new_spk = []
                    for m in range(MC):
                        msl = slice(m * P, (m + 1) * P)
                        ps = pp.tile([P, NW], F32, tag="mm", bufs=4)
                        first, mm = True, []
                        # collect (lhsT, rhs) in emission order
                        if i == 0:
                            for k in range(KC):
                                mm.append((wff0_sb[k][:, msl], rhs_ff[k]))
                            for s in range(NS):
                                for k in range(KC):
                                    mm.append((wfb3_sb[0][s][k][:, msl], rhs_fb[k]))
                        elif i == 1:
                            for s in range(NS):
                                for k in range(KC):
                                    mm.append((wff3_sb[0][s][k][:, msl], rhs_ff[k]))
                                    mm.append((wfb3_sb[1][s][k][:, msl], rhs_fb[k]))
                        else:
                            for k in range(KC):
                                mm.append((wfb2_sb[k][:, msl], rhs_fb[k]))
                            for s in range(NS):
                                for k in range(KC):
                                    mm.append((wff3_sb[1][s][k][:, msl], rhs_ff[k]))
                        for j, (lw, rr) in enumerate(mm):
                            last = (j == len(mm) - 1) and not use_bias
                            nc.tensor.matmul(ps[:], lw, rr[:], start=(j == 0),
                                             stop=last)
                        if use_bias:
                            nc.tensor.matmul(ps[:], bc_sb[i][0:1, msl], ones[0:1, :],
                                             start=False, stop=True)
                        # ps = 0.1*(ff+fb) [+ 0.1*(b_ff+b_fb)]

                        dend = sp.tile([P, NW], F32, tag="dend", bufs=3)
                        nc.sync.dma_start(dend[:], dendT[i, msl, ns])
                        soma = sp.tile([P, NW], F32, tag="soma", bufs=3)
                        nc.sync.dma_start(soma[:], somaT[i, msl, ns])
                        bst = sp.tile([P, NW], F32, tag="bst", bufs=3)
                        nc.sync.dma_start(bst[:], bT[i, msl, ns])
                        sh = spk_cur[m]

                        # u9 = 0.9*(1 - spikes)
                        u = sp.tile([P, NW], F32, tag="u", bufs=2)
                        nc.scalar.activation(u[:], sh[:], AF.Copy,
                                             bias=float(ALPHA_M), scale=-float(ALPHA_M))
                        # a_new = 0.9*dend + ps
                        anew = sp.tile([P, NW], F32, tag="anew", bufs=3)
                        nc.vector.scalar_tensor_tensor(
                            anew[:], dend[:], float(ALPHA_A), ps[:], OP.mult, OP.add)
                        # m9 = soma * u9
                        m9 = sp.tile([P, NW], F32, tag="m9", bufs=2)
                        nc.gpsimd.tensor_mul(m9[:], soma[:], u[:])
                        # sm = 0.1*a_new + m9
                        smt = sp.tile([P, NW], F32, tag="smt", bufs=3)
                        nc.vector.scalar_tensor_tensor(
                            smt[:], anew[:], float(ONE_MINUS_AM), m9[:], OP.mult, OP.add)
                        # s04 = 0.04*spikes
                        s04 = sp.tile([P, NW], F32, tag="s04", bufs=2)
                        nc.scalar.activation(s04[:], sh[:], AF.Copy,
                                             scale=float(ONE_MINUS_RHO))
                        # bb = 0.96*b + s04
                        bbt = sp.tile([P, NW], F32, tag="bbt", bufs=3)
                        nc.vector.scalar_tensor_tensor(
                            bbt[:], bst[:], float(RHO), s04[:], OP.mult, OP.add)
                        # v = -1.8*bb + sm ; spk = v > 0.1  (bf16, exact 0/1)
                        v = sp.tile([P, NW], F32, tag="v", bufs=2)
                        nc.vector.scalar_tensor_tensor(
                            v[:], bbt[:], -float(BETA), smt[:], OP.mult, OP.add)
                        spk = sp.tile([P, NW], BF16, tag="spk", bufs=8)
                        nc.vector.tensor_single_scalar(spk[:], v[:], float(B0), OP.is_gt)

                        nc.sync.dma_start(outT[i, msl, ns], smt[:])
                        nc.sync.dma_start(outT[L + i, msl, ns], anew[:])
                        nc.sync.dma_start(outT[2 * L + i, msl, ns], bbt[:])
                        nc.sync.dma_start(outSpkT[i, msl, ns], spk[:])
                        new_spk.append(spk)

                    rhs_ff = new_spk
                    if i + 1 < L:
                        spk_cur = spk_next

                # ---- readout update: 0.9*readout + spk2 @ W_out.T + b_out ----
                if n == 0:
                    for k in range(KC):
                        for s in range(NS):
                            nc.sync.dma_start(wout3_sb[s][k][:],
                                              wout3[s, k * P:(k + 1) * P, :])
                    if use_bias:
                        nc.sync.dma_start(bo_sb[:], boutD[:, :])
                for m in range(MC):
                    msl = slice(m * P, (m + 1) * P)
                    psr = pp.tile([P, NW], F32, tag="mm", bufs=4)
                    j = 0
                    for s in range(NS):
                        for k in range(KC):
                            last = (j == NS * KC - 1) and not use_bias
                            nc.tensor.matmul(psr[:], wout3_sb[s][k][:, msl],
                                             rhs_ff[k][:], start=(j == 0), stop=last)
                            j += 1
                    if use_bias:
                        nc.tensor.matmul(psr[:], bo_sb[0:1, msl], ones[0:1, :],
                                         start=False, stop=True)
                    routt = sp.tile([P, NW], F32, tag="rout", bufs=2)
                    nc.vector.scalar_tensor_tensor(
                        routt[:], read_sb[m][:], float(ALPHA_OUT), psr[:],
                        OP.mult, OP.add)
                    nc.sync.dma_start(outT[3 * L, msl, ns], routt[:])

    nc.compile()
    return nc


def _split3_bf16(w):
    """Exact 3-way bf16 split of an fp32 array: w == s[0]+s[1]+s[2] (fp32 sum)."""
    w = np.asarray(w, np.float32)
    w1 = w.astype(NP_BF16)
    r1 = w - w1.astype(np.float32)
    w2 = r1.astype(NP_BF16)
    r2 = r1 - w2.astype(np.float32)
    w3 = r2.astype(NP_BF16)
    return np.stack([w1, w2, w3])


def make_in_maps(x, soma, spikes_h, dendrites, b, readout,
                 W_ff, b_ff, W_fb, b_fb, W_out, b_out):
    """Shard + transpose inputs; fold scalar prefactors into weights."""
    f32 = np.float32
    x = np.asarray(x, f32)
    soma = np.asarray(soma, f32)
    spikes_h = np.asarray(spikes_h, f32)
    dendrites = np.asarray(dendrites, f32)
    b = np.asarray(b, f32)
    readout = np.asarray(readout, f32)
    W_ff = np.asarray(W_ff, f32)
    b_ff = np.asarray(b_ff, f32)
    W_fb = np.asarray(W_fb, f32)
    b_fb = np.asarray(b_fb, f32)
    W_out = np.asarray(W_out, f32)
    b_out = np.asarray(b_out, f32)

    # effective (transposed) weights with 0.1 = 1-ALPHA_A folded in; layer-0 ff
    # also folds the 0.5 input scale
    wffTe = [np.ascontiguousarray(
        (W_ff[i] * (ONE_MINUS_AA * (f32(0.5) if i == 0 else f32(1.0)))).T)
        for i in range(L)]
    wfbTe = [np.ascontiguousarray((W_fb[i] * ONE_MINUS_AA).T) for i in range(L)]
    woutTe = np.ascontiguousarray(W_out.T)

    wff0T = wffTe[0]
    wfb2T = wfbTe[2]
    wff3 = np.ascontiguousarray(np.stack([_split3_bf16(wffTe[1]),
                                          _split3_bf16(wffTe[2])]))
    wfb3 = np.ascontiguousarray(np.stack([_split3_bf16(wfbTe[0]),
                                          _split3_bf16(wfbTe[1])]))
    wout3 = np.ascontiguousarray(_split3_bf16(woutTe))
    bcombA = np.ascontiguousarray(
        (ONE_MINUS_AA * (b_ff + b_fb)).reshape(L, 1, D))
    boutA = np.ascontiguousarray(b_out.reshape(1, D))

    in_maps = []
    for c in range(NCORES):
        sl = slice(c * BL, (c + 1) * BL)
        in_maps.append({
            "xT": np.ascontiguousarray(x[sl].T),
            "somaT": np.ascontiguousarray(soma[:, sl, :].transpose(0, 2, 1)),
            "spikesT": np.ascontiguousarray(
                spikes_h[:, sl, :].transpose(0, 2, 1)).astype(NP_BF16),
            "dendT": np.ascontiguousarray(dendrites[:, sl, :].transpose(0, 2, 1)),
            "bT": np.ascontiguousarray(b[:, sl, :].transpose(0, 2, 1)),
            "readT": np.ascontiguousarray(readout[sl].T),
            "wff0T": wff0T,
            "wfb2T": wfb2T,
            "wff3": wff3,
            "wfb3": wfb3,
            "wout3": wout3,
            "bcomb": bcombA,
            "boutD": boutA,
        })
    return in_maps


def assemble_output(results):
    """[10,D,BL] f32 + [3,D,BL] bf16 per core -> [13, B, D] f32."""
    out = np.empty((4 * L + 1, B, D), np.float32)
    for c in range(NCORES):
        sl = slice(c * BL, (c + 1) * BL)
        r, spk = results[c]["outT"], results[c]["outSpkT"]
        for i in range(L):
            out[i, sl, :] = r[i].T                      # sm
            out[L + i, sl, :] = spk[i].astype(np.float32).T   # spikes
            out[2 * L + i, sl, :] = r[L + i].T          # a_new
            out[3 * L + i, sl, :] = r[2 * L + i].T      # bb
        out[4 * L, sl, :] = r[3 * L].T                  # readout_new
    return out


_CACHE = {}


def _get_program(use_bias=False):
    key = ("nc", use_bias)
    if key not in _CACHE:
        _CACHE[key] = build_program(use_bias)
    return _CACHE[key]


def kernel(**inputs):
    use_bias = bool(np.any(inputs["b_ff"]) or np.any(inputs["b_fb"])
                    or np.any(inputs["b_out"]))
    nc = _get_program(use_bias)
    in_maps = make_in_maps(**inputs)
    res = run_bass_kernel_spmd(nc, in_maps, core_ids=list(range(NCORES)))
    return assemble_output(res.results)


# revision 10
# speedup vs baseline: 1.3917x; 1.2744x over previous
"""EnergySNN single-step kernel for Trainium2, 8-core data parallel.

Reference computation (per batch row, D=512, L=3 layers):
    s = 0.5*x
    for i in 0..2:
        fb_in = spikes_h[i+1]            (i<2)   |  readout/||readout||  (i==2)
        ff = s @ W_ff[i].T + b_ff[i]
        fb = fb_in @ W_fb[i].T + b_fb[i]
        a_new = 0.9*dend[i] + 0.1*(ff+fb)
        sm    = 0.9*soma[i]*(1-spikes_h[i]) + 0.1*a_new
        bb    = 0.96*b[i] + 0.04*spikes_h[i]
        spk   = (sm - (0.1 + 1.8*bb)) > 0
        s = spk
    readout_new = 0.9*readout + s @ W_out.T + b_out
    out = [sm(3), spk(3), a_new(3), bb(3), readout_new(1)]  -> [13, B, D]

Strategy: pure data parallel over batch (8192 -> 8 x 1024). All [B,D]
activations/state are held in TRANSPOSED layout [D, B_local] on device so that
the matmul moving operand (rhs, contraction over D on partitions) and the
elementwise state updates share one layout -- no on-device transposes, fully
contiguous DMA. Host does the (cheap) numpy transposes and folds the scalar
prefactors 0.5 (input scale) and 0.1 (=1-ALPHA_A) into the weights.

fp32 matmul runs at 4 PE-cycles/row (two half-rate passes). For the 5 GEMMs
whose moving operand is exact in bf16 (spike vectors in {0,1}), the fp32
weights are split exactly into three bf16 matrices (W = W1+W2+W3 covering all
24 mantissa bits); bf16xbf16 products are exact and accumulate in fp32 PSUM,
giving fp32-accurate results at 3 cycles/row. Spikes move as bf16 (exact).

The two 512-column batch chunks are interleaved through the layer loop so the
PE always has independent work while a layer's spike outputs (needed as the
next layer's moving operand) flow through the vector-engine chain. DMA issue
is spread over three sequencers (sync: weights/activations, vector: state
loads, scalar: output stores) to avoid serializing on one issue queue.
"""

import numpy as np
import sys

sys.path.insert(0, "/opt/trn_rl_repo")

import concourse.bass as bass
import concourse.bacc as bacc
import concourse.mybir as mybir
from concourse import tile
from concourse.bass_utils import run_bass_kernel_spmd

F32 = mybir.dt.float32
BF16 = mybir.dt.bfloat16
NP_BF16 = mybir.dt.np(BF16)
OP = mybir.AluOpType
AF = mybir.ActivationFunctionType

# Problem constants (hardcoded per contract)
B = 8192
D = 512
L = 3
NCORES = 8
BL = B // NCORES          # 1024 batch rows per core
P = 128                   # partitions
KC = D // P               # 4 contraction chunks
MC = D // P               # 4 output-d chunks
NW = 512                  # free-dim chunk width (one PSUM bank of fp32)
NCH = BL // NW            # 2 n-chunks per core
NS = 3                    # bf16 splits per fp32 weight

ALPHA_M = np.float32(0.9)
ALPHA_A = np.float32(0.9)
RHO = np.float32(0.96)
BETA = np.float32(1.8)
B0 = np.float32(0.1)
ALPHA_OUT = np.float32(0.9)
EPS = np.float32(1e-12)
ONE_MINUS_AM = np.float32(1.0) - ALPHA_M      # 0.1
ONE_MINUS_AA = np.float32(1.0) - ALPHA_A      # 0.1
ONE_MINUS_RHO = np.float32(1.0) - RHO         # 0.04


def build_program(use_bias=False):
    """Build the per-core SPMD Bass/Tile program."""
    nc = bacc.Bacc("TRN2", target_bir_lowering=False)

    # --- DRAM I/O (per-core shapes, transposed world) ---
    xT = nc.dram_tensor("xT", [D, BL], F32, kind="ExternalInput")
    somaT = nc.dram_tensor("somaT", [L, D, BL], F32, kind="ExternalInput")
    spikesT = nc.dram_tensor("spikesT", [L, D, BL], BF16, kind="ExternalInput")
    dendT = nc.dram_tensor("dendT", [L, D, BL], F32, kind="ExternalInput")
    bT = nc.dram_tensor("bT", [L, D, BL], F32, kind="ExternalInput")
    readT = nc.dram_tensor("readT", [D, BL], F32, kind="ExternalInput")
    # fp32 weights: layer-0 ff (x rhs), layer-2 fb (normalized-readout rhs)
    wff0T = nc.dram_tensor("wff0T", [D, D], F32, kind="ExternalInput")
    wfb2T = nc.dram_tensor("wfb2T", [D, D], F32, kind="ExternalInput")
    # bf16 3-way exact splits: ff layers 1,2 / fb layers 0,1 / out
    wff3 = nc.dram_tensor("wff3", [2, NS, D, D], BF16, kind="ExternalInput")
    wfb3 = nc.dram_tensor("wfb3", [2, NS, D, D], BF16, kind="ExternalInput")
    wout3 = nc.dram_tensor("wout3", [NS, D, D], BF16, kind="ExternalInput")
    bcomb = nc.dram_tensor("bcomb", [L, 1, D], F32, kind="ExternalInput")
    boutD = nc.dram_tensor("boutD", [1, D], F32, kind="ExternalInput")
    # f32 outputs: sm(0-2), a_new(3-5), bb(6-8), readout_new(9)
    outT = nc.dram_tensor("outT", [3 * L + 1, D, BL], F32, kind="ExternalOutput")
    # spikes out, bf16 (exact 0/1)
    outSpkT = nc.dram_tensor("outSpkT", [L, D, BL], BF16, kind="ExternalOutput")

    ld_w = nc.sync       # all loads
    ld_st = nc.sync      # state loads
    st = nc.scalar       # output stores

    with tile.TileContext(nc) as tc:
        with (
            tc.tile_pool(name="wpool", bufs=1) as wp,
            tc.tile_pool(name="spool", bufs=2) as sp,
            tc.tile_pool(name="ppool", bufs=1, space=bass.MemorySpace.PSUM) as pp,
        ):
            # ---- constants ----
            ones128 = wp.tile([P, 1], F32, tag="ones128")
            nc.vector.memset(ones128[:], 1.0)
            ones = wp.tile([1, P], F32, tag="ones")
            nc.vector.memset(ones[:], 1.0)
            onesN = wp.tile([1, NW], F32, tag="onesN")
            nc.vector.memset(onesN[:], 1.0)

            # weight tiles: one WIDE tile per [D,D] matrix, k-chunks in the
            # free dim (cols k*D + m*P...), loaded in a single DMA.
            def wload_f32(name, src2d):
                t = wp.tile([P, KC * D], F32, tag="wf32w", bufs=2, name=name)
                ld_w.dma_start(t[:].rearrange("p (k n) -> p k n", k=KC),
                               src2d.rearrange("(k p) n -> p k n", p=P))
                return t

            def wload_bf16(name, src2d):
                t = wp.tile([P, KC * D], BF16, tag="wbf16w", bufs=8, name=name)
                ld_w.dma_start(t[:].rearrange("p (k n) -> p k n", k=KC),
                               src2d.rearrange("(k p) n -> p k n", p=P))
                return t

            def wsl(t, k, msl):
                # lhsT [P, 128] for contraction chunk k, output chunk msl
                return t[:, k * D + msl.start: k * D + msl.stop]

            bc_sb = [wp.tile([1, D], F32, tag=f"bc{i}", name=f"bc{i}")
                     for i in range(L)]
            bo_sb = wp.tile([1, D], F32, tag="bo")

            def load_weights(i):
                """Allocate + DMA layer i's weights (just before first use).
                Returns (ff_tiles, fb_tiles) lists over splits."""
                if i == 0:
                    ff = [wload_f32("wff0", wff0T[:, :])]
                    fb = [wload_bf16(f"wfb3_0_{s}", wfb3[0, s]) for s in range(NS)]
                elif i == 1:
                    ff = [wload_bf16(f"wff3_0_{s}", wff3[0, s]) for s in range(NS)]
                    fb = [wload_bf16(f"wfb3_1_{s}", wfb3[1, s]) for s in range(NS)]
                else:
                    ff = [wload_bf16(f"wff3_1_{s}", wff3[1, s]) for s in range(NS)]
                    fb = [wload_f32("wfb2", wfb2T[:, :])]
                if use_bias:
                    ld_w.dma_start(bc_sb[i][:], bcomb[i, :, :])
                return ff, fb

            # ---- prologue per n-chunk: x, readout, norm chain ----
            rhs_ff = {}    # n -> list over k of rhs tiles for current layer's ff
            fbin = {}      # n -> fbin tiles (layer-2 fb rhs)
            read_sb = {}   # n -> readout tiles
            spk_cur = {}   # n -> spikes_h[i] tiles for current layer
            nsl = [slice(n * NW, (n + 1) * NW) for n in range(NCH)]

            wl0 = load_weights(0)
            for n in range(NCH):
                xs = []
                for k in range(KC):
                    t = sp.tile([P, NW], F32, tag="xs", bufs=8)
                    ld_w.dma_start(t[:], xT[k * P:(k + 1) * P, nsl[n]])
                    xs.append(t)
                rhs_ff[n] = xs
                sc = []
                for k in range(KC):
                    t = sp.tile([P, NW], BF16, tag="spkh", bufs=12)
                    ld_w.dma_start(t[:], spikesT[0, k * P:(k + 1) * P, nsl[n]])
                    sc.append(t)
                spk_cur[n] = sc

            for n in range(NCH):
                # normalized readout: nrm over partition dim via PE
                rsb = []
                for k in range(KC):
                    t = sp.tile([P, NW], F32, tag="read", bufs=8)
                    ld_w.dma_start(t[:], readT[k * P:(k + 1) * P, nsl[n]])
                    rsb.append(t)
                read_sb[n] = rsb
                psum_n = pp.tile([1, NW], F32, tag="pn", bufs=2)
                for k in range(KC):
                    sq = sp.tile([P, NW], F32, tag="sq", bufs=2)
                    nc.scalar.activation(sq[:], rsb[k][:], AF.Square)
                    nc.tensor.matmul(psum_n[:], ones128[:, 0:1], sq[:],
                                     start=(k == 0), stop=(k == KC - 1))
                nrm = sp.tile([1, NW], F32, tag="nrm", bufs=2)
                nc.scalar.activation(nrm[:], psum_n[:], AF.Sqrt)
                nrm2 = sp.tile([1, NW], F32, tag="nrm2", bufs=2)
                nc.vector.tensor_scalar_max(nrm2[:], nrm[:], float(EPS))
                rn = sp.tile([1, NW], F32, tag="rn", bufs=2)
                nc.vector.reciprocal(rn[:], nrm2[:])
                psum_b = pp.tile([P, NW], F32, tag="pb", bufs=2)
                nc.tensor.matmul(psum_b[:], ones[0:1, :], rn[:],
                                 start=True, stop=True)
                fbn = []
                for k in range(KC):
                    t = sp.tile([P, NW], F32, tag="fbin", bufs=8)
                    nc.vector.tensor_mul(t[:], rsb[k][:], psum_b[:])
                    fbn.append(t)
                fbin[n] = fbn

            # ---- layer loop, n-chunks interleaved ----
            for i in range(L):
                wff_i, wfb_i = wl0 if i == 0 else load_weights(i)
                for n in range(NCH):
                    ns = nsl[n]
                    # fb rhs for this layer
                    if i + 1 < L:
                        spk_next = []
                        for k in range(KC):
                            t = sp.tile([P, NW], BF16, tag="spkh", bufs=12)
                            ld_w.dma_start(
                                t[:], spikesT[i + 1, k * P:(k + 1) * P, ns])
                            spk_next.append(t)
                        rhs_fb = spk_next
                    else:
                        rhs_fb = fbin[n]

                    new_spk = []
                    for m in range(MC):
                        msl = slice(m * P, (m + 1) * P)
                        ps = pp.tile([P, NW], F32, tag="mm", bufs=4)
                        mm = []
                        if i == 0:
                            for k in range(KC):
                                mm.append((wsl(wff_i[0], k, msl), rhs_ff[n][k]))
                            for s in range(NS):
                                for k in range(KC):
                                    mm.append((wsl(wfb_i[s], k, msl), rhs_fb[k]))
                        elif i == 1:
                            for s in range(NS):
                                for k in range(KC):
                                    mm.append((wsl(wff_i[s], k, msl), rhs_ff[n][k]))
                                    mm.append((wsl(wfb_i[s], k, msl), rhs_fb[k]))
                        else:
                            for k in range(KC):
                                mm.append((wsl(wfb_i[0], k, msl), rhs_fb[k]))
                            for s in range(NS):
                                for k in range(KC):
                                    mm.append((wsl(wff_i[s], k, msl), rhs_ff[n][k]))
                        for j, (lw, rr) in enumerate(mm):
                            last = (j == len(mm) - 1) and not use_bias
                            nc.tensor.matmul(ps[:], lw, rr[:], start=(j == 0),
                                             stop=last)
                        if use_bias:
                            nc.tensor.matmul(ps[:], bc_sb[i][0:1, msl],
                                             onesN[0:1, :], start=False, stop=True)
                        # ps = 0.1*(ff+fb) [+ 0.1*(b_ff+b_fb)]

                        dend = sp.tile([P, NW], F32, tag="dend", bufs=3)
                        ld_st.dma_start(dend[:], dendT[i, msl, ns])
                        soma = sp.tile([P, NW], F32, tag="soma", bufs=3)
                        ld_st.dma_start(soma[:], somaT[i, msl, ns])
                        bst = sp.tile([P, NW], F32, tag="bst", bufs=3)
                        ld_st.dma_start(bst[:], bT[i, msl, ns])
                        sh = spk_cur[n][m]

                        # u9 = 0.9*(1 - spikes)
                        u = sp.tile([P, NW], F32, tag="u", bufs=2)
                        nc.scalar.activation(u[:], sh[:], AF.Copy,
                                             bias=float(ALPHA_M), scale=-float(ALPHA_M))
                        # a_new = 0.9*dend + ps
                        anew = sp.tile([P, NW], F32, tag="anew", bufs=3)
                        nc.vector.scalar_tensor_tensor(
                            anew[:], dend[:], float(ALPHA_A), ps[:], OP.mult, OP.add)
                        # m9 = soma * u9
                        m9 = sp.tile([P, NW], F32, tag="m9", bufs=2)
                        nc.gpsimd.tensor_mul(m9[:], soma[:], u[:])
                        # sm = 0.1*a_new + m9
                        smt = sp.tile([P, NW], F32, tag="smt", bufs=3)
                        nc.vector.scalar_tensor_tensor(
                            smt[:], anew[:], float(ONE_MINUS_AM), m9[:], OP.mult, OP.add)
                        # s04 = 0.04*spikes
                        s04 = sp.tile([P, NW], F32, tag="s04", bufs=2)
                        nc.scalar.activation(s04[:], sh[:], AF.Copy,
                                             scale=float(ONE_MINUS_RHO))
                        # bb = 0.96*b + s04
                        bbt = sp.tile([P, NW], F32, tag="bbt", bufs=3)
                        nc.vector.scalar_tensor_tensor(
                            bbt[:], bst[:], float(RHO), s04[:], OP.mult, OP.add)
                        # v = -1.8*bb + sm ; spk = v > 0.1  (bf16, exact 0/1)
                        v = sp.tile([P, NW], F32, tag="v", bufs=2)
                        nc.vector.scalar_tensor_tensor(
                            v[:], bbt[:], -float(BETA), smt[:], OP.mult, OP.add)
                        spk = sp.tile([P, NW], BF16, tag="spk", bufs=12)
                        nc.vector.tensor_single_scalar(spk[:], v[:], float(B0),
                                                       OP.is_gt)

                        st.dma_start(outT[i, msl, ns], smt[:])
                        st.dma_start(outT[L + i, msl, ns], anew[:])
                        st.dma_start(outT[2 * L + i, msl, ns], bbt[:])
                        st.dma_start(outSpkT[i, msl, ns], spk[:])
                        new_spk.append(spk)

                    rhs_ff[n] = new_spk
                    if i + 1 < L:
                        spk_cur[n] = spk_next

            # ---- readout update: 0.9*readout + spk2 @ W_out.T + b_out ----
            wout_sb = [wload_bf16(f"wout3_{s}", wout3[s]) for s in range(NS)]
            if use_bias:
                ld_w.dma_start(bo_sb[:], boutD[:, :])
            for n in range(NCH):
                ns = nsl[n]
                for m in range(MC):
                    msl = slice(m * P, (m + 1) * P)
                    psr = pp.tile([P, NW], F32, tag="mm", bufs=4)
                    j = 0
                    for s in range(NS):
                        for k in range(KC):
                            last = (j == NS * KC - 1) and not use_bias
                            nc.tensor.matmul(psr[:], wsl(wout_sb[s], k, msl),
                                             rhs_ff[n][k][:], start=(j == 0),
                                             stop=last)
                            j += 1
                    if use_bias:
                        nc.tensor.matmul(psr[:], bo_sb[0:1, msl], onesN[0:1, :],
                                         start=False, stop=True)
                    routt = sp.tile([P, NW], F32, tag="rout", bufs=2)
                    nc.vector.scalar_tensor_tensor(
                        routt[:], read_sb[n][m][:], float(ALPHA_OUT), psr[:],
                        OP.mult, OP.add)
                    st.dma_start(outT[3 * L, msl, ns], routt[:])

    nc.compile()
    return nc


def _split3_bf16(w):
    """Exact 3-way bf16 split of an fp32 array: w == s[0]+s[1]+s[2] (fp32 sum)."""
    w = np.asarray(w, np.float32)
    w1 = w.astype(NP_BF16)
    r1 = w - w1.astype(np.float32)
    w2 = r1.astype(NP_BF16)
    r2 = r1 - w2.astype(np.float32)
    w3 = r2.astype(NP_BF16)
    return np.stack([w1, w2, w3])


def make_in_maps(x, soma, spikes_h, dendrites, b, readout,
                 W_ff, b_ff, W_fb, b_fb, W_out, b_out):
    """Shard + transpose inputs; fold scalar prefactors into weights."""
    f32 = np.float32
    x = np.asarray(x, f32)
    soma = np.asarray(soma, f32)
    spikes_h = np.asarray(spikes_h, f32)
    dendrites = np.asarray(dendrites, f32)
    b = np.asarray(b, f32)
    readout = np.asarray(readout, f32)
    W_ff = np.asarray(W_ff, f32)
    b_ff = np.asarray(b_ff, f32)
    W_fb = np.asarray(W_fb, f32)
    b_fb = np.asarray(b_fb, f32)
    W_out = np.asarray(W_out, f32)
    b_out = np.asarray(b_out, f32)

    # effective (transposed) weights with 0.1 = 1-ALPHA_A folded in; layer-0 ff
    # also folds the 0.5 input scale
    wffTe = [np.ascontiguousarray(
        (W_ff[i] * (ONE_MINUS_AA * (f32(0.5) if i == 0 else f32(1.0)))).T)
        for i in range(L)]
    wfbTe = [np.ascontiguousarray((W_fb[i] * ONE_MINUS_AA).T) for i in range(L)]
    woutTe = np.ascontiguousarray(W_out.T)

    wff0T = wffTe[0]
    wfb2T = wfbTe[2]
    wff3 = np.ascontiguousarray(np.stack([_split3_bf16(wffTe[1]),
                                          _split3_bf16(wffTe[2])]))
    wfb3 = np.ascontiguousarray(np.stack([_split3_bf16(wfbTe[0]),
                                          _split3_bf16(wfbTe[1])]))
    wout3 = np.ascontiguousarray(_split3_bf16(woutTe))
    bcombA = np.ascontiguousarray(
        (ONE_MINUS_AA * (b_ff + b_fb)).reshape(L, 1, D))
    boutA = np.ascontiguousarray(b_out.reshape(1, D))

    in_maps = []
    for c in range(NCORES):
        sl = slice(c * BL, (c + 1) * BL)
        in_maps.append({
            "xT": np.ascontiguousarray(x[sl].T),
            "somaT": np.ascontiguousarray(soma[:, sl, :].transpose(0, 2, 1)),
            "spikesT": np.ascontiguousarray(
                spikes_h[:, sl, :].transpose(0, 2, 1)).astype(NP_BF16),
            "dendT": np.ascontiguousarray(dendrites[:, sl, :].transpose(0, 2, 1)),
            "bT": np.ascontiguousarray(b[:, sl, :].transpose(0, 2, 1)),
            "readT": np.ascontiguousarray(readout[sl].T),
            "wff0T": wff0T,
            "wfb2T": wfb2T,
            "wff3": wff3,
            "wfb3": wfb3,
            "wout3": wout3,
            "bcomb": bcombA,
            "boutD": boutA,
        })
    return in_maps


def assemble_output(results):
    """[10,D,BL] f32 + [3,D,BL] bf16 per core -> [13, B, D] f32."""
    out = np.empty((4 * L + 1, B, D), np.float32)
    for c in range(NCORES):
        sl = slice(c * BL, (c + 1) * BL)
        r, spk = results[c]["outT"], results[c]["outSpkT"]
        for i in range(L):
            out[i, sl, :] = r[i].T                      # sm
            out[L + i, sl, :] = spk[i].astype(np.float32).T   # spikes
            out[2 * L + i, sl, :] = r[L + i].T          # a_new
            out[3 * L + i, sl, :] = r[2 * L + i].T      # bb
        out[4 * L, sl, :] = r[3 * L].T                  # readout_new
    return out


_CACHE = {}


def _get_program(use_bias=False):
    key = ("nc", use_bias)
    if key not in _CACHE:
        _CACHE[key] = build_program(use_bias)
    return _CACHE[key]


def kernel(**inputs):
    use_bias = bool(np.any(inputs["b_ff"]) or np.any(inputs["b_fb"])
                    or np.any(inputs["b_out"]))
    nc = _get_program(use_bias)
    in_maps = make_in_maps(**inputs)
    res = run_bass_kernel_spmd(nc, in_maps, core_ids=list(range(NCORES)))
    return assemble_output(res.results)


# revision 16
# speedup vs baseline: 1.4383x; 1.0334x over previous
"""EnergySNN single-step kernel for Trainium2, 8-core data parallel.

Reference computation (per batch row, D=512, L=3 layers):
    s = 0.5*x
    for i in 0..2:
        fb_in = spikes_h[i+1]            (i<2)   |  readout/||readout||  (i==2)
        ff = s @ W_ff[i].T + b_ff[i]
        fb = fb_in @ W_fb[i].T + b_fb[i]
        a_new = 0.9*dend[i] + 0.1*(ff+fb)
        sm    = 0.9*soma[i]*(1-spikes_h[i]) + 0.1*a_new
        bb    = 0.96*b[i] + 0.04*spikes_h[i]
        spk   = (sm - (0.1 + 1.8*bb)) > 0
        s = spk
    readout_new = 0.9*readout + s @ W_out.T + b_out
    out = [sm(3), spk(3), a_new(3), bb(3), readout_new(1)]  -> [13, B, D]

Strategy: pure data parallel over batch (8192 -> 8 x 1024). All [B,D]
activations/state are held in TRANSPOSED layout [D, B_local] on device so that
the matmul moving operand (rhs, contraction over D on partitions) and the
elementwise state updates share one layout -- no on-device transposes, fully
contiguous DMA. Host does the (cheap) numpy transposes and folds the scalar
prefactors 0.5 (input scale) and 0.1 (=1-ALPHA_A) into the weights.

fp32 matmul runs at 4 PE-cycles/row (two half-rate passes). For the 5 GEMMs
whose moving operand is exact in bf16 (spike vectors in {0,1}), the fp32
weights are split exactly into three bf16 matrices (W = W1+W2+W3 covering all
24 mantissa bits); bf16xbf16 products are exact and accumulate in fp32 PSUM,
giving fp32-accurate results at 3 cycles/row. Spikes move as bf16 (exact).

The two 512-column batch chunks are interleaved through the layer loop so the
PE always has independent work while a layer's spike outputs (needed as the
next layer's moving operand) flow through the vector-engine chain. DMA issue
is split across two sequencers (sync: all loads, scalar: output stores), each
weight matrix loads as one wide-tile DMA, and layer i+1's weights are
prefetched one n-chunk early to keep the PE gap-free at layer boundaries.
"""

import numpy as np
import sys

sys.path.insert(0, "/opt/trn_rl_repo")

import concourse.bass as bass
import concourse.bacc as bacc
import concourse.mybir as mybir
from concourse import tile
from concourse.bass_utils import run_bass_kernel_spmd

F32 = mybir.dt.float32
BF16 = mybir.dt.bfloat16
NP_BF16 = mybir.dt.np(BF16)
OP = mybir.AluOpType
AF = mybir.ActivationFunctionType

# Problem constants (hardcoded per contract)
B = 8192
D = 512
L = 3
NCORES = 8
BL = B // NCORES          # 1024 batch rows per core
P = 128                   # partitions
KC = D // P               # 4 contraction chunks
MC = D // P               # 4 output-d chunks
NW = 512                  # free-dim chunk width (one PSUM bank of fp32)
NCH = BL // NW            # 2 n-chunks per core
NS = 3                    # bf16 splits per fp32 weight

ALPHA_M = np.float32(0.9)
ALPHA_A = np.float32(0.9)
RHO = np.float32(0.96)
BETA = np.float32(1.8)
B0 = np.float32(0.1)
ALPHA_OUT = np.float32(0.9)
EPS = np.float32(1e-12)
ONE_MINUS_AM = np.float32(1.0) - ALPHA_M      # 0.1
ONE_MINUS_AA = np.float32(1.0) - ALPHA_A      # 0.1
ONE_MINUS_RHO = np.float32(1.0) - RHO         # 0.04


def build_program(use_bias=False):
    """Build the per-core SPMD Bass/Tile program."""
    nc = bacc.Bacc("TRN2", target_bir_lowering=False)

    # --- DRAM I/O (per-core shapes, transposed world) ---
    xT = nc.dram_tensor("xT", [D, BL], F32, kind="ExternalInput")
    somaT = nc.dram_tensor("somaT", [L, D, BL], F32, kind="ExternalInput")
    spikesT = nc.dram_tensor("spikesT", [L, D, BL], BF16, kind="ExternalInput")
    dendT = nc.dram_tensor("dendT", [L, D, BL], F32, kind="ExternalInput")
    bT = nc.dram_tensor("bT", [L, D, BL], F32, kind="ExternalInput")
    readT = nc.dram_tensor("readT", [D, BL], F32, kind="ExternalInput")
    # fp32 weights: layer-0 ff (x rhs), layer-2 fb (normalized-readout rhs)
    wff0T = nc.dram_tensor("wff0T", [D, D], F32, kind="ExternalInput")
    wfb2T = nc.dram_tensor("wfb2T", [D, D], F32, kind="ExternalInput")
    # bf16 3-way exact splits: ff layers 1,2 / fb layers 0,1 / out
    wff3 = nc.dram_tensor("wff3", [2, NS, D, D], BF16, kind="ExternalInput")
    wfb3 = nc.dram_tensor("wfb3", [2, NS, D, D], BF16, kind="ExternalInput")
    wout3 = nc.dram_tensor("wout3", [NS, D, D], BF16, kind="ExternalInput")
    bcomb = nc.dram_tensor("bcomb", [L, 1, D], F32, kind="ExternalInput")
    boutD = nc.dram_tensor("boutD", [1, D], F32, kind="ExternalInput")
    # f32 outputs: sm(0-2), a_new(3-5), bb(6-8), readout_new(9)
    outT = nc.dram_tensor("outT", [3 * L + 1, D, BL], F32, kind="ExternalOutput")
    # spikes out, bf16 (exact 0/1)
    outSpkT = nc.dram_tensor("outSpkT", [L, D, BL], BF16, kind="ExternalOutput")

    ld_w = nc.sync       # all loads
    ld_st = nc.sync      # state loads
    st = nc.scalar       # output stores

    with tile.TileContext(nc) as tc:
        with (
            tc.tile_pool(name="wpool", bufs=1) as wp,
            tc.tile_pool(name="spool", bufs=2) as sp,
            tc.tile_pool(name="ppool", bufs=1, space=bass.MemorySpace.PSUM) as pp,
        ):
            # ---- constants ----
            ones128 = wp.tile([P, 1], F32, tag="ones128")
            nc.vector.memset(ones128[:], 1.0)
            ones = wp.tile([1, P], F32, tag="ones")
            nc.vector.memset(ones[:], 1.0)
            onesN = wp.tile([1, NW], F32, tag="onesN")
            nc.vector.memset(onesN[:], 1.0)

            # weight tiles: one WIDE tile per [D,D] matrix, k-chunks in the
            # free dim (cols k*D + m*P...), loaded in a single DMA.
            def wload_f32(name, src2d):
                t = wp.tile([P, KC * D], F32, tag="wf32w", bufs=2, name=name)
                ld_w.dma_start(t[:].rearrange("p (k n) -> p k n", k=KC),
                               src2d.rearrange("(k p) n -> p k n", p=P))
                return t

            def wload_bf16(name, src2d):
                t = wp.tile([P, KC * D], BF16, tag="wbf16w", bufs=9, name=name)
                ld_w.dma_start(t[:].rearrange("p (k n) -> p k n", k=KC),
                               src2d.rearrange("(k p) n -> p k n", p=P))
                return t

            def wsl(t, k, msl):
                # lhsT [P, 128] for contraction chunk k, output chunk msl
                return t[:, k * D + msl.start: k * D + msl.stop]

            bc_sb = [wp.tile([1, D], F32, tag=f"bc{i}", name=f"bc{i}")
                     for i in range(L)]
            bo_sb = wp.tile([1, D], F32, tag="bo")

            def load_weights(i):
                """Allocate + DMA layer i's weights (just before first use).
                Returns (ff_tiles, fb_tiles) lists over splits."""
                if i == 0:
                    ff = [wload_f32("wff0", wff0T[:, :])]
                    fb = [wload_bf16(f"wfb3_0_{s}", wfb3[0, s]) for s in range(NS)]
                elif i == 1:
                    ff = [wload_bf16(f"wff3_0_{s}", wff3[0, s]) for s in range(NS)]
                    fb = [wload_bf16(f"wfb3_1_{s}", wfb3[1, s]) for s in range(NS)]
                else:
                    ff = [wload_bf16(f"wff3_1_{s}", wff3[1, s]) for s in range(NS)]
                    fb = [wload_f32("wfb2", wfb2T[:, :])]
                if use_bias:
                    ld_w.dma_start(bc_sb[i][:], bcomb[i, :, :])
                return ff, fb

            # ---- prologue per n-chunk: x, readout, norm chain ----
            rhs_ff = {}    # n -> list over k of rhs tiles for current layer's ff
            fbin = {}      # n -> fbin tiles (layer-2 fb rhs)
            read_sb = {}   # n -> readout tiles
            spk_cur = {}   # n -> spikes_h[i] tiles for current layer
            nsl = [slice(n * NW, (n + 1) * NW) for n in range(NCH)]

            wl0 = load_weights(0)
            for n in range(NCH):
                xs = []
                for k in range(KC):
                    t = sp.tile([P, NW], F32, tag="xs", bufs=8)
                    ld_w.dma_start(t[:], xT[k * P:(k + 1) * P, nsl[n]])
                    xs.append(t)
                rhs_ff[n] = xs
                sc = []
                for k in range(KC):
                    t = sp.tile([P, NW], BF16, tag="spkh", bufs=12)
                    ld_w.dma_start(t[:], spikesT[0, k * P:(k + 1) * P, nsl[n]])
                    sc.append(t)
                spk_cur[n] = sc

            for n in range(NCH):
                # normalized readout: nrm over partition dim via PE
                rsb = []
                for k in range(KC):
                    t = sp.tile([P, NW], F32, tag="read", bufs=8)
                    ld_w.dma_start(t[:], readT[k * P:(k + 1) * P, nsl[n]])
                    rsb.append(t)
                read_sb[n] = rsb
                psum_n = pp.tile([1, NW], F32, tag="pn", bufs=2)
                for k in range(KC):
                    sq = sp.tile([P, NW], F32, tag="sq", bufs=1)
                    nc.scalar.activation(sq[:], rsb[k][:], AF.Square)
                    nc.tensor.matmul(psum_n[:], ones128[:, 0:1], sq[:],
                                     start=(k == 0), stop=(k == KC - 1))
                nrm = sp.tile([1, NW], F32, tag="nrm", bufs=2)
                nc.scalar.activation(nrm[:], psum_n[:], AF.Sqrt)
                nrm2 = sp.tile([1, NW], F32, tag="nrm2", bufs=2)
                nc.vector.tensor_scalar_max(nrm2[:], nrm[:], float(EPS))
                rn = sp.tile([1, NW], F32, tag="rn", bufs=2)
                nc.vector.reciprocal(rn[:], nrm2[:])
                psum_b = pp.tile([P, NW], F32, tag="pb", bufs=2)
                nc.tensor.matmul(psum_b[:], ones[0:1, :], rn[:],
                                 start=True, stop=True)
                fbn = []
                for k in range(KC):
                    t = sp.tile([P, NW], F32, tag="fbin", bufs=8)
                    nc.vector.tensor_mul(t[:], rsb[k][:], psum_b[:])
                    fbn.append(t)
                fbin[n] = fbn

            # ---- layer loop, n-chunks interleaved ----
            wnext = {0: wl0}
            for i in range(L):
                if i not in wnext:
                    wnext[i] = load_weights(i)
                wff_i, wfb_i = wnext[i]
                for n in range(NCH):
                    if n == 1 and i + 1 == L - 1:
                        wnext[i + 1] = load_weights(i + 1)
                    ns = nsl[n]
                    # fb rhs for this layer
                    if i + 1 < L:
                        spk_next = []
                        for k in range(KC):
                            t = sp.tile([P, NW], BF16, tag="spkh", bufs=12)
                            ld_w.dma_start(
                                t[:], spikesT[i + 1, k * P:(k + 1) * P, ns])
                            spk_next.append(t)
                        rhs_fb = spk_next
                    else:
                        rhs_fb = fbin[n]

                    new_spk = []
                    for m in range(MC):
                        msl = slice(m * P, (m + 1) * P)
                        ps = pp.tile([P, NW], F32, tag="mm", bufs=4)
                        mm = []
                        if i == 0:
                            for k in range(KC):
                                mm.append((wsl(wff_i[0], k, msl), rhs_ff[n][k]))
                            for s in range(NS):
                                for k in range(KC):
                                    mm.append((wsl(wfb_i[s], k, msl), rhs_fb[k]))
                        elif i == 1:
                            for s in range(NS):
                                for k in range(KC):
                                    mm.append((wsl(wff_i[s], k, msl), rhs_ff[n][k]))
                                    mm.append((wsl(wfb_i[s], k, msl), rhs_fb[k]))
                        else:
                            for k in range(KC):
                                mm.append((wsl(wfb_i[0], k, msl), rhs_fb[k]))
                            for s in range(NS):
                                for k in range(KC):
                                    mm.append((wsl(wff_i[s], k, msl), rhs_ff[n][k]))
                        for j, (lw, rr) in enumerate(mm):
                            last = (j == len(mm) - 1) and not use_bias
                            nc.tensor.matmul(ps[:], lw, rr[:], start=(j == 0),
                                             stop=last)
                        if use_bias:
                            nc.tensor.matmul(ps[:], bc_sb[i][0:1, msl],
                                             onesN[0:1, :], start=False, stop=True)
                        # ps = 0.1*(ff+fb) [+ 0.1*(b_ff+b_fb)]

                        dend = sp.tile([P, NW], F32, tag="dend", bufs=3)
                        ld_st.dma_start(dend[:], dendT[i, msl, ns])
                        soma = sp.tile([P, NW], F32, tag="soma", bufs=3)
                        ld_st.dma_start(soma[:], somaT[i, msl, ns])
                        bst = sp.tile([P, NW], F32, tag="bst", bufs=3)
                        ld_st.dma_start(bst[:], bT[i, msl, ns])
                        sh = spk_cur[n][m]

                        # u9 = 0.9*(1 - spikes)
                        u = sp.tile([P, NW], F32, tag="u", bufs=2)
                        nc.scalar.activation(u[:], sh[:], AF.Copy,
                                             bias=float(ALPHA_M), scale=-float(ALPHA_M))
                        # a_new = 0.9*dend + ps
                        anew = sp.tile([P, NW], F32, tag="anew", bufs=3)
                        nc.vector.scalar_tensor_tensor(
                            anew[:], dend[:], float(ALPHA_A), ps[:], OP.mult, OP.add)
                        # m9 = soma * u9
                        m9 = sp.tile([P, NW], F32, tag="m9", bufs=2)
                        nc.gpsimd.tensor_mul(m9[:], soma[:], u[:])
                        # sm = 0.1*a_new + m9
                        smt = sp.tile([P, NW], F32, tag="smt", bufs=3)
                        nc.vector.scalar_tensor_tensor(
                            smt[:], anew[:], float(ONE_MINUS_AM), m9[:], OP.mult, OP.add)
                        # s04 = 0.04*spikes
                        s04 = sp.tile([P, NW], F32, tag="s04", bufs=2)
                        nc.scalar.activation(s04[:], sh[:], AF.Copy,
                                             scale=float(ONE_MINUS_RHO))
                        # bb = 0.96*b + s04
                        bbt = sp.tile([P, NW], F32, tag="bbt", bufs=3)
                        nc.vector.scalar_tensor_tensor(
                            bbt[:], bst[:], float(RHO), s04[:], OP.mult, OP.add)
                        # v = -1.8*bb + sm ; spk = v > 0.1  (bf16, exact 0/1)
                        v = sp.tile([P, NW], F32, tag="v", bufs=2)
                        nc.vector.scalar_tensor_tensor(
                            v[:], bbt[:], -float(BETA), smt[:], OP.mult, OP.add)
                        spk = sp.tile([P, NW], BF16, tag="spk", bufs=12)
                        nc.vector.tensor_single_scalar(spk[:], v[:], float(B0),
                                                       OP.is_gt)

                        st.dma_start(outT[i, msl, ns], smt[:])
                        st.dma_start(outT[L + i, msl, ns], anew[:])
                        st.dma_start(outT[2 * L + i, msl, ns], bbt[:])
                        st.dma_start(outSpkT[i, msl, ns], spk[:])
                        new_spk.append(spk)

                    rhs_ff[n] = new_spk
                    if i + 1 < L:
                        spk_cur[n] = spk_next

            # ---- readout update: 0.9*readout + spk2 @ W_out.T + b_out ----
            wout_sb = [wload_bf16(f"wout3_{s}", wout3[s]) for s in range(NS)]
            if use_bias:
                ld_w.dma_start(bo_sb[:], boutD[:, :])
            for n in range(NCH):
                ns = nsl[n]
                for m in range(MC):
                    msl = slice(m * P, (m + 1) * P)
                    psr = pp.tile([P, NW], F32, tag="mm", bufs=4)
                    j = 0
                    for s in range(NS):
                        for k in range(KC):
                            last = (j == NS * KC - 1) and not use_bias
                            nc.tensor.matmul(psr[:], wsl(wout_sb[s], k, msl),
                                             rhs_ff[n][k][:], start=(j == 0),
                                             stop=last)
                            j += 1
                    if use_bias:
                        nc.tensor.matmul(psr[:], bo_sb[0:1, msl], onesN[0:1, :],
                                         start=False, stop=True)
                    routt = sp.tile([P, NW], F32, tag="rout", bufs=2)
                    nc.vector.scalar_tensor_tensor(
                        routt[:], read_sb[n][m][:], float(ALPHA_OUT), psr[:],
                        OP.mult, OP.add)
                    st.dma_start(outT[3 * L, msl, ns], routt[:])

    nc.compile()
    return nc


def _split3_bf16(w):
    """Exact 3-way bf16 split of an fp32 array: w == s[0]+s[1]+s[2] (fp32 sum)."""
    w = np.asarray(w, np.float32)
    w1 = w.astype(NP_BF16)
    r1 = w - w1.astype(np.float32)
    w2 = r1.astype(NP_BF16)
    r2 = r1 - w2.astype(np.float32)
    w3 = r2.astype(NP_BF16)
    return np.stack([w1, w2, w3])


def make_in_maps(x, soma, spikes_h, dendrites, b, readout,
                 W_ff, b_ff, W_fb, b_fb, W_out, b_out):
    """Shard + transpose inputs; fold scalar prefactors into weights."""
    f32 = np.float32
    x = np.asarray(x, f32)
    soma = np.asarray(soma, f32)
    spikes_h = np.asarray(spikes_h, f32)
    dendrites = np.asarray(dendrites, f32)
    b = np.asarray(b, f32)
    readout = np.asarray(readout, f32)
    W_ff = np.asarray(W_ff, f32)
    b_ff = np.asarray(b_ff, f32)
    W_fb = np.asarray(W_fb, f32)
    b_fb = np.asarray(b_fb, f32)
    W_out = np.asarray(W_out, f32)
    b_out = np.asarray(b_out, f32)

    # effective (transposed) weights with 0.1 = 1-ALPHA_A folded in; layer-0 ff
    # also folds the 0.5 input scale
    wffTe = [np.ascontiguousarray(
        (W_ff[i] * (ONE_MINUS_AA * (f32(0.5) if i == 0 else f32(1.0)))).T)
        for i in range(L)]
    wfbTe = [np.ascontiguousarray((W_fb[i] * ONE_MINUS_AA).T) for i in range(L)]
    woutTe = np.ascontiguousarray(W_out.T)

    wff0T = wffTe[0]
    wfb2T = wfbTe[2]
    wff3 = np.ascontiguousarray(np.stack([_split3_bf16(wffTe[1]),
                                          _split3_bf16(wffTe[2])]))
    wfb3 = np.ascontiguousarray(np.stack([_split3_bf16(wfbTe[0]),
                                          _split3_bf16(wfbTe[1])]))
    wout3 = np.ascontiguousarray(_split3_bf16(woutTe))
    bcombA = np.ascontiguousarray(
        (ONE_MINUS_AA * (b_ff + b_fb)).reshape(L, 1, D))
    boutA = np.ascontiguousarray(b_out.reshape(1, D))

    in_maps = []
    for c in range(NCORES):
        sl = slice(c * BL, (c + 1) * BL)
        in_maps.append({
            "xT": np.ascontiguousarray(x[sl].T),
            "somaT": np.ascontiguousarray(soma[:, sl, :].transpose(0, 2, 1)),
            "spikesT": np.ascontiguousarray(
                spikes_h[:, sl, :].transpose(0, 2, 1)).astype(NP_BF16),
            "dendT": np.ascontiguousarray(dendrites[:, sl, :].transpose(0, 2, 1)),
            "bT": np.ascontiguousarray(b[:, sl, :].transpose(0, 2, 1)),
            "readT": np.ascontiguousarray(readout[sl].T),
            "wff0T": wff0T,
            "wfb2T": wfb2T,
            "wff3": wff3,
            "wfb3": wfb3,
            "wout3": wout3,
            "bcomb": bcombA,
            "boutD": boutA,
        })
    return in_maps


def assemble_output(results):
    """[10,D,BL] f32 + [3,D,BL] bf16 per core -> [13, B, D] f32."""
    out = np.empty((4 * L + 1, B, D), np.float32)
    for c in range(NCORES):
        sl = slice(c * BL, (c + 1) * BL)
        r, spk = results[c]["outT"], results[c]["outSpkT"]
        for i in range(L):
            out[i, sl, :] = r[i].T                      # sm
            out[L + i, sl, :] = spk[i].astype(np.float32).T   # spikes
            out[2 * L + i, sl, :] = r[L + i].T          # a_new
            out[3 * L + i, sl, :] = r[2 * L + i].T      # bb
        out[4 * L, sl, :] = r[3 * L].T                  # readout_new
    return out


_CACHE = {}


def _get_program(use_bias=False):
    key = ("nc", use_bias)
    if key not in _CACHE:
        _CACHE[key] = build_program(use_bias)
    return _CACHE[key]


def kernel(**inputs):
    use_bias = bool(np.any(inputs["b_ff"]) or np.any(inputs["b_fb"])
                    or np.any(inputs["b_out"]))
    nc = _get_program(use_bias)
    in_maps = make_in_maps(**inputs)
    res = run_bass_kernel_spmd(nc, in_maps, core_ids=list(range(NCORES)))
    return assemble_output(res.results)
